# revision 18
# baseline (speedup 1.0000x reference)
"""Trainium2 Bass kernel for nn_Actor (dense MLP trunk + 64 softmax heads).

Data-parallel over 8 NeuronCores: batch 4096 -> 512 rows/core, weights
replicated. Feature-major trunk (activations [features, batch]) so layer
outputs feed the next contraction without transposes; heads run batch-major
so per-head softmax reduces along the free dim.

Precision: trunk layers AND heads run fp8-e4m3 DoubleRow matmuls (256-deep
contraction per instruction; weights pre-scaled x256, x pre-scaled x32,
h2 stored as 32*h2 in fp8 -- all compensated via activation scale=).
Head bias rides in the contraction (k-tiles 8..9 are constants so the
extra DR pair adds 8192*b to every logit row).  Softmax post-processing
runs in bf16 (exp -> bf16, grouped reduces / reciprocal / normalize all
bf16 on DVE 2x mode); the output is written to DRAM in bf16 and upcast
to f32 on the host.

PE warm-up: ~12 dummy FD=512 matmuls on a zeroed SBUF tile run first so
the PE HAM clock-gate reaches 8/8 (2.4 GHz) before the first real matmul
(otherwise the whole L1 ramp runs at 1.2 GHz).

DMA: weights SBUF-resident; the L1 ramp (m0..m3, k-pair-major) gets its
w1 halves k-staged across sync (even m) and gpsimd (odd m) rings so
arrival order matches PE consumption order; xt + biases stream on scalar.
The last two k-pairs of the ramp run m-major so ACT's relu of m0 overlaps
the remaining ramp matmuls and m4 never waits on a PSUM bank.  All
constant memsets run on vector (idle early) so gpsimd's ring starts with
its first w1 transfer immediately.

Tail: the final batch-tile's last head-pair is processed chunk-wise with
all-vector normalize (gpsimd's tensor ops are ~2.5x slower) and output
DMAs on sync/gpsimd only -- never scalar, so the final exp is not
head-of-line blocked behind a waiting DMA issue.

Self-contained: hardcodes shapes; host-side prep packs head weights into one
[1024, 1280] fp8 GEMM whose columns are already in the final output order
(per vehicle v: rsu[2v] | rsu[2v+1] | lay[2v] | lay[2v+1]).
"""

import os
import numpy as np

B, IN_DIM, HIDDEN, H2 = 4096, 2048, 2048, 1024
V, R, L = 16, 32, 8
OUTC = V * (2 * R + 2 * L)          # 1280
NCORES = 8
BC = B // NCORES                    # 512 batch rows per core
KT1 = IN_DIM // 128                 # 16 k-tiles, layer 1
MT1 = HIDDEN // 128                 # 16 m-tiles, layer 1
KT2 = HIDDEN // 128                 # 16 k-tiles, layer 2
MT2 = H2 // 128                     # 8 m-tiles, layer 2
KTH = H2 // 128                     # 8 k-tiles, heads
BT = BC // 128                      # 4 batch tiles per core
CW = 320                            # head chunk width = 4 vehicles
NCH = OUTC // CW                    # 4 chunks
VC = CW // (2 * (R + L))            # 4 vehicles per chunk

_CACHE = {}
LAST_RESULTS = None                 # BassKernelResults from the last run


def _build():
    import concourse.bacc as bacc
    import concourse.mybir as mybir
    import concourse.tile as tile

    F32 = mybir.dt.float32
    BF16 = mybir.dt.bfloat16
    F8 = mybir.dt.float8e4
    DR = mybir.MatmulPerfMode.DoubleRow
    Relu = mybir.ActivationFunctionType.Relu
    Exp = mybir.ActivationFunctionType.Exp
    X = mybir.AxisListType.X

    nc = bacc.Bacc("TRN2", target_bir_lowering=False, debug=False,
                   num_devices=NCORES)

    xt = nc.dram_tensor("xt", [128, KT1, BC], F8, kind="ExternalInput")
    w1t = nc.dram_tensor("w1t", [128, MT1, KT1, 128], F8,
                         kind="ExternalInput")
    b1c = nc.dram_tensor("b1c", [128, MT1], F32, kind="ExternalInput")
    w2t = nc.dram_tensor("w2t", [128, MT2, KT2, 128], F8,
                         kind="ExternalInput")
    b2c = nc.dram_tensor("b2c", [128, MT2], F32, kind="ExternalInput")
    wht = nc.dram_tensor("wht", [128, KTH + 1, OUTC], F8,
                         kind="ExternalInput")
    out = nc.dram_tensor("out", [BC, OUTC], BF16, kind="ExternalOutput")

    with tile.TileContext(nc) as tc:
        with (
            tc.tile_pool(name="const", bufs=1) as cp,
            tc.tile_pool(name="sm", bufs=8) as sp,
            tc.tile_pool(name="ps", bufs=4, space="PSUM") as ps,
            tc.tile_pool(name="psh", bufs=2, space="PSUM") as psh,
        ):
            xt_sb = cp.tile([128, KT1, BC], F8, tag="xt")
            h1_sb = cp.tile([128, KT2, BC], F8, tag="h1")
            h2_sb = cp.tile([128, KTH + 2, BC], F8, tag="h2")
            wh_sb = cp.tile([128, KTH + 2, OUTC], F8, tag="wh")
            b1_sb = cp.tile([128, MT1], F32, tag="b1")
            b2_sb = cp.tile([128, MT2], F32, tag="b2")
            w1_sb = cp.tile([128, MT1, KT1, 128], F8, tag="w1")
            w2_sb = cp.tile([128, MT2, KT2, 128], F8, tag="w2")
            warm_sb = cp.tile([128, 512], F8, tag="warm")

            def w1ap(m, k0, k1):
                return w1_sb[:, m:m + 1, k0:k1, :].rearrange(
                    "p a k c -> p (a k) c")

            def w2ap(m, k0, k1):
                return w2_sb[:, m:m + 1, k0:k1, :].rearrange(
                    "p a k c -> p (a k) c")

            # --- PE warm-up: dummy matmuls on zeros so the HAM clock gate
            # opens (4/8 -> 8/8) before the first data-dependent matmul.
            # gpsimd memset: it runs right after the framework preamble,
            # ~1us before vector clears its barrier.
            nc.gpsimd.memset(warm_sb[:], 0.0)
            wps = ps.tile([128, BC], F32, tag="acc")
            for _ in range(9):
                nc.tensor.matmul(wps[:], warm_sb[:, 0:128],
                                 warm_sb[:], start=True, stop=True)

            # bias-in-contraction constants (vector is idle early):
            # k-tiles 8..9 of h2 (partition 0 of k=8 holds 32.0, rest zero)
            # so the extra DR pair adds 32*256*bh = 8192*bh[c] per logit.
            nc.vector.memset(h2_sb[:, KTH:KTH + 2, :], 0.0)
            nc.vector.memset(h2_sb[0:1, KTH, :], 32.0)
            nc.vector.memset(wh_sb[:, KTH + 1, :], 0.0)

            # --- DMA descriptors.  All rings share one ~250-350 GB/s HBM
            # pipe with packet-level round-robin, so per-ring queue ORDER is
            # the only priority control: each ring carries small consumption-
            # ordered chunks, with bulk (w2/wh) queued behind all of w1.
            # scalar: only xt + biases -- its queue must drain before the
            # L1 relus (same engine queue) start at ~19us.
            nc.scalar.dma_start(xt_sb[:, 0:2, :], xt.ap()[:, 0:2, :])
            nc.scalar.dma_start(xt_sb[:, 2:4, :], xt.ap()[:, 2:4, :])
            nc.scalar.dma_start(xt_sb[:, 4:8, :], xt.ap()[:, 4:8, :])
            nc.scalar.dma_start(b1_sb[:], b1c.ap())
            nc.scalar.dma_start(b2_sb[:], b2c.ap())

            # sync: even ramp tiles k-staged, even m-major tiles, then bulk.
            # gpsimd mirrors with the odd tiles.
            def wdma(eng, m, k0, k1):
                eng.dma_start(w1_sb[:, m:m + 1, k0:k1, :],
                              w1t.ap()[:, m:m + 1, k0:k1, :])

            wdma(nc.sync, 0, 0, 8)
            wdma(nc.sync, 2, 0, 8)
            wdma(nc.sync, 3, 0, 8)
            nc.sync.dma_start(xt_sb[:, 8:12, :], xt.ap()[:, 8:12, :])
            wdma(nc.sync, 0, 8, 12)
            wdma(nc.sync, 3, 8, 12)
            wdma(nc.sync, 0, 12, 16)
            wdma(nc.sync, 3, 12, 16)
            wdma(nc.gpsimd, 1, 0, 8)
            nc.gpsimd.dma_start(xt_sb[:, 12:16, :], xt.ap()[:, 12:16, :])
            wdma(nc.gpsimd, 1, 8, 12)
            wdma(nc.gpsimd, 2, 8, 12)
            wdma(nc.gpsimd, 1, 12, 16)
            wdma(nc.gpsimd, 2, 12, 16)
            for m in range(4, MT1):
                eng = nc.sync if m % 2 == 0 else nc.gpsimd
                wdma(eng, m, 0, 8)
                wdma(eng, m, 8, 16)
            for m in range(MT2):
                eng = nc.sync if m % 2 == 0 else nc.gpsimd
                eng.dma_start(w2_sb[:, m:m + 1, :, :],
                              w2t.ap()[:, m:m + 1, :, :])
            nc.sync.dma_start(wh_sb[:, 0:5, :], wht.ap()[:, 0:5, :])
            nc.gpsimd.dma_start(wh_sb[:, 5:KTH + 1, :],
                                wht.ap()[:, 5:KTH + 1, :])

            # --- Layer 1: h1[m] = relu(sum_k w1[k,m].T @ xt[k] + b1[m]) ---
            # Ramp m0..3 k-pair-major on 4 PSUM banks so PE consumption
            # tracks chunk arrivals; the last two k-pairs run m-major so
            # relu(m0) overlaps the ramp tail and m4 never waits on a bank.
            RM = 4
            raccs = [ps.tile([128, BC], F32, name=f"racc{i}", tag="acc")
                     for i in range(RM)]
            for k in range(0, KT1 - 4, 2):
                for mi in (0, 2, 1, 3):
                    nc.tensor.matmul(raccs[mi][:], w1ap(mi, k, k + 2),
                                     xt_sb[:, k:k + 2, :],
                                     start=(k == 0), stop=False, perf_mode=DR)
            for mi in range(RM):
                for k in range(KT1 - 4, KT1, 2):
                    nc.tensor.matmul(raccs[mi][:], w1ap(mi, k, k + 2),
                                     xt_sb[:, k:k + 2, :],
                                     start=False, stop=(k == KT1 - 2),
                                     perf_mode=DR)
                nc.scalar.activation(h1_sb[:, mi, :], raccs[mi][:], Relu,
                                     bias=b1_sb[:, mi:mi + 1],
                                     scale=1.0 / 512.0)
            for m in range(RM, MT1):
                acc = ps.tile([128, BC], F32, tag="acc")
                for k in range(0, KT1, 2):
                    nc.tensor.matmul(acc[:], w1ap(m, k, k + 2),
                                     xt_sb[:, k:k + 2, :],
                                     start=(k == 0), stop=(k == KT1 - 2),
                                     perf_mode=DR)
                nc.scalar.activation(h1_sb[:, m, :], acc[:], Relu,
                                     bias=b1_sb[:, m:m + 1], scale=1.0 / 512.0)

            # --- Layer 2: h2[m] = relu(sum_k w2[k,m].T @ h1[k] + b2[m]) ---
            for m in range(MT2):
                acc = ps.tile([128, BC], F32, tag="acc")
                for k in range(0, KT2, 2):
                    nc.tensor.matmul(acc[:], w2ap(m, k, k + 2),
                                     h1_sb[:, k:k + 2, :],
                                     start=(k == 0), stop=(k == KT2 - 2),
                                     perf_mode=DR)
                nc.scalar.activation(h2_sb[:, m, :], acc[:], Relu,
                                     bias=b2_sb[:, m:m + 1], scale=1.0 / 128.0)

            # --- Heads: logits = h2.T @ wh in fp8 DoubleRow (bias rides in
            # k-tiles 8..9), then softmax in bf16.
            def reduces(et, w, sdst, reng=None):
                # grouped softmax sums: rsu groups (32-wide) and lay groups
                # (8-wide) into sdst [128, 4*w*VC]
                reng = reng or nc.vector
                PW = w * CW
                VP = w * VC
                nv = et[:, 0:PW].rearrange("p (v x) -> p v x", v=VP)
                rsu4 = nv[:, :, 0:2 * R].rearrange("p v (h c) -> p v h c", h=2)
                lay4 = nv[:, :, 2 * R:].rearrange("p v (h c) -> p v h c", h=2)
                s_r = sdst[:, 0:2 * VP].rearrange("p (v h) -> p v h", h=2)
                s_l = sdst[:, 2 * VP:4 * VP].rearrange(
                    "p (v h) -> p v h", h=2)
                with nc.allow_low_precision(reason="bf16 softmax sums"):
                    reng.reduce_sum(out=s_r.unsqueeze(3), in_=rsu4, axis=X)
                    reng.reduce_sum(out=s_l.unsqueeze(3), in_=lay4, axis=X)

            def recip(dst, srcv):
                with nc.allow_low_precision(reason="bf16 softmax recip"):
                    nc.vector.reciprocal(dst, srcv)

            def norm(et, c0, w, rsrc, oeng, rmeng, lmeng, oeng2=None):
                # normalize: rsu block on rmeng, lay block on lmeng
                PW = w * CW
                VP = w * VC
                nv = et[:, 0:PW].rearrange("p (v x) -> p v x", v=VP)
                rsu4 = nv[:, :, 0:2 * R].rearrange("p v (h c) -> p v h c", h=2)
                lay4 = nv[:, :, 2 * R:].rearrange("p v (h c) -> p v h c", h=2)
                o_sb = sp.tile([128, 2 * CW], BF16, tag="o")
                ov = o_sb[:, 0:PW].rearrange("p (v x) -> p v x", v=VP)
                orsu = ov[:, :, 0:2 * R].rearrange("p v (h c) -> p v h c", h=2)
                olay = ov[:, :, 2 * R:].rearrange("p v (h c) -> p v h c", h=2)
                r_r = rsrc[:, 0:2 * VP].rearrange("p (v h) -> p v h", h=2)
                r_l = rsrc[:, 2 * VP:4 * VP].rearrange(
                    "p (v h) -> p v h", h=2)
                rmeng.tensor_mul(
                    orsu, rsu4,
                    r_r.unsqueeze(3).broadcast_to([128, VP, 2, R]))
                lmeng.tensor_mul(
                    olay, lay4,
                    r_l.unsqueeze(3).broadcast_to([128, VP, 2, L]))
                if oeng2 is not None:
                    # tail: DMA the rsu and lay blocks separately so each
                    # leaves as soon as its multiply lands
                    RB = VP * 2 * R
                    oeng.dma_start(out.ap()[bsl, c0:c0 + RB], o_sb[:, 0:RB])
                    oeng2.dma_start(out.ap()[bsl, c0 + RB:c0 + PW],
                                    o_sb[:, RB:PW])
                else:
                    oeng.dma_start(out.ap()[bsl, c0:c0 + PW], o_sb[:, 0:PW])

            pidx = 0
            for bt in range(BT):
                bsl = slice(bt * 128, (bt + 1) * 128)
                last_bt = bt == BT - 1
                if not last_bt:
                    sums_bt = sp.tile([128, 64], BF16, tag="sums")
                    rec_bt = sp.tile([128, 64], BF16, tag="rec")
                for pr in range(NCH // 2):
                    accs = []
                    if pidx % 2 == 0:
                        for ci in range(2):
                            hacc = psh.tile([128, CW], F32, tag=f"hacc{ci}")
                            accs.append(hacc)
                    else:
                        # odd pairs borrow the (now idle) trunk PSUM banks so
                        # four pairs are in flight
                        for ci in range(2):
                            hacc = ps.tile([128, BC], F32, tag="acc")
                            accs.append(hacc[:, 0:CW])
                    for k in range(0, KTH + 2, 2):
                        for ci in range(2):
                            c = 2 * pr + ci
                            nc.tensor.matmul(accs[ci][:],
                                             h2_sb[:, k:k + 2, bsl],
                                             wh_sb[:, k:k + 2,
                                                   c * CW:(c + 1) * CW],
                                             start=(k == 0), stop=(k == KTH),
                                             perf_mode=DR)
                    c0 = 2 * pr * CW
                    if not last_bt:
                        et = sp.tile([128, 2 * CW], BF16, tag="et")
                        for ci in range(2):
                            nc.scalar.activation(et[:, ci * CW:(ci + 1) * CW],
                                                 accs[ci][:], Exp,
                                                 scale=1.0 / 8192.0)
                        reduces(et, 2, sums_bt[:, pr * 32:pr * 32 + 32])
                        recip(rec_bt[:, pr * 32:pr * 32 + 32],
                                             sums_bt[:, pr * 32:pr * 32 + 32])
                        norm(et, c0, 2, rec_bt[:, pr * 32:pr * 32 + 32],
                             oeng=nc.sync, rmeng=nc.gpsimd, lmeng=nc.vector)
                    elif pr == 0:
                        et = sp.tile([128, 2 * CW], BF16, tag="et")
                        for ci in range(2):
                            nc.scalar.activation(et[:, ci * CW:(ci + 1) * CW],
                                                 accs[ci][:], Exp,
                                                 scale=1.0 / 8192.0)
                        sums0 = sp.tile([128, 64], BF16, tag="sums")
                        rec0 = sp.tile([128, 64], BF16, tag="rec")
                        reduces(et, 2, sums0[:, 0:32])
                        recip(rec0[:, 0:32], sums0[:, 0:32])
                        norm(et, c0, 2, rec0[:, 0:32],
                             oeng=nc.sync, rmeng=nc.gpsimd, lmeng=nc.vector)
                    else:
                        # final pair chunk-wise for the shortest tail chain
                        sa = sp.tile([128, 64], BF16, tag="sums")
                        ra = sp.tile([128, 64], BF16, tag="rec")
                        eta = sp.tile([128, CW], BF16, tag="eta")
                        nc.scalar.activation(eta[:], accs[0][:], Exp,
                                             scale=1.0 / 8192.0)
                        reduces(eta, 1, sa[:, 0:16])
                        recip(ra[:, 0:16], sa[:, 0:16])
                        norm(eta, c0, 1, ra[:, 0:16],
                             oeng=nc.scalar, rmeng=nc.gpsimd,
                             lmeng=nc.gpsimd, oeng2=nc.scalar)
                        etb = sp.tile([128, CW], BF16, tag="etb")
                        nc.scalar.activation(etb[:], accs[1][:], Exp,
                                             scale=1.0 / 8192.0)
                        reduces(etb, 1, sa[:, 32:48])
                        recip(ra[:, 32:48], sa[:, 32:48])
                        norm(etb, c0 + CW, 1, ra[:, 32:48],
                             oeng=nc.sync, rmeng=nc.vector, lmeng=nc.vector,
                             oeng2=nc.sync)
                    pidx += 1

    nc.compile()
    return nc


def _prep_shared(w1, b1, w2, b2, w_rsu, b_rsu, w_lay, b_lay):
    import ml_dtypes
    f = np.float32
    f8 = ml_dtypes.float8_e4m3
    w1t = np.ascontiguousarray(
        np.clip(w1 * 256.0, -240, 240).astype(f8)
        .reshape(KT1, 128, MT1, 128).transpose(1, 2, 0, 3))
    w2t = np.ascontiguousarray(
        np.clip(w2 * 256.0, -240, 240).astype(f8)
        .reshape(KT2, 128, MT2, 128).transpose(1, 2, 0, 3))
    b1c = np.ascontiguousarray(16.0 * b1.reshape(MT1, 128).T, dtype=f)
    b2c = np.ascontiguousarray(32.0 * b2.reshape(MT2, 128).T, dtype=f)

    wh = np.empty((H2, OUTC), dtype=f)
    bh = np.empty((OUTC,), dtype=f)
    for v in range(V):
        c = 2 * (R + L) * v
        wh[:, c:c + R] = w_rsu[2 * v]
        wh[:, c + R:c + 2 * R] = w_rsu[2 * v + 1]
        wh[:, c + 2 * R:c + 2 * R + L] = w_lay[2 * v]
        wh[:, c + 2 * R + L:c + 2 * (R + L)] = w_lay[2 * v + 1]
        bh[c:c + R] = b_rsu[2 * v]
        bh[c + R:c + 2 * R] = b_rsu[2 * v + 1]
        bh[c + 2 * R:c + 2 * R + L] = b_lay[2 * v]
        bh[c + 2 * R + L:c + 2 * (R + L)] = b_lay[2 * v + 1]
    whx = np.zeros((KTH + 1, 128, OUTC), dtype=f)
    whx[0:KTH] = (wh * 256.0).reshape(KTH, 128, OUTC)
    whx[KTH, 0, :] = 256.0 * bh
    wht = np.ascontiguousarray(
        np.clip(whx, -240, 240).astype(f8).transpose(1, 0, 2))
    return {"w1t": w1t, "b1c": b1c, "w2t": w2t, "b2c": b2c,
            "wht": wht}


def kernel(x, w1, b1, w2, b2, w_rsu, b_rsu, w_lay, b_lay):
    global LAST_RESULTS
    import ml_dtypes
    from concourse.bass_utils import run_bass_kernel_spmd

    if "nc" not in _CACHE:
        _CACHE["nc"] = _build()
    nc = _CACHE["nc"]

    shared = _prep_shared(np.asarray(w1, np.float32), np.asarray(b1, np.float32),
                          np.asarray(w2, np.float32), np.asarray(b2, np.float32),
                          np.asarray(w_rsu, np.float32), np.asarray(b_rsu, np.float32),
                          np.asarray(w_lay, np.float32), np.asarray(b_lay, np.float32))

    # x [B, IN] -> per-core xt [128, KT1, BC] with [p, k, n] = x[core*BC+n, k*128+p]
    # fp8 e4m3 with x*32 so small values clear the subnormal range; the
    # combined 32*256 scale comes out in the L1 relu (scale=1/512 -> 16*h1)
    xt_full = np.clip(np.ascontiguousarray(np.asarray(x, np.float32).T) * 32.0,
                      -240, 240) \
        .astype(ml_dtypes.float8_e4m3).reshape(KT1, 128, B).transpose(1, 0, 2)
    in_maps = []
    for c in range(NCORES):
        m = dict(shared)
        m["xt"] = np.ascontiguousarray(xt_full[:, :, c * BC:(c + 1) * BC])
        in_maps.append(m)

    trace = os.environ.get("KERNEL_TRACE", "") == "1"
    LAST_RESULTS = run_bass_kernel_spmd(nc, in_maps, core_ids=list(range(NCORES)),
                                        trace=trace)
    return np.concatenate([r["out"] for r in LAST_RESULTS.results],
                          axis=0).astype(np.float32)
